# revision 15
# baseline (speedup 1.0000x reference)
"""DeepClusterLoss on 8 Trainium2 NeuronCores (Bass/Tile).

reference:
    recon_loss   = sum((recon_x - x)**2)
    cluster_loss = sum((x - centers[assign])**2)
    total        = recon_loss + cluster_loss          (ALPHA = BETA = 1)

Strategy (data-parallel over N):
  - Shard N over 8 cores.  Within a core, HOST-side index prep sorts the
    shard by cluster assignment and pads every cluster's run to a multiple
    of 128, so each 128-sample "slot" belongs to exactly one cluster
    (pads are all-zero rows that contribute nothing to any sum).
  - Inputs stream as fp8 e3m4 (max 15.5, 4 mantissa bits: N(0,1) data fits
    with ~1e-4 statistical bias on the quadratic sums; tolerance is 2e-2).
    Layout per tile: [128 part, 32 slots, 128 cols] where cols = [x | r].
  - Everything quadratic in r rides the Tensor engine: per slot,
    lhsT = [x|r] (128 cols, fast-weight-load eligible) and
       MM_A: rhs = r-half    -> [128,64] psum, accumulated over ALL slots:
             rows 0:64  = x^T r  (diag sum  -> sum <x_i, r_i>)
             rows 64:128= r^T r  (diag sum  -> sum r^2)
       MM_B: rhs = ones[128,1]-> per-slot column sums (rows 0:64 = slot
             sums of x).  Host groups slot sums into S_k (its cluster is
             known from the sort), giving the cluster cross term
             sum_k <S_k, C_k> without ever materializing a one-hot.
  - sum x^2 is computed by ACT (Square + accum_out) on most tiles and by
    DVE (mult + reduce) on the rest, balancing the two engines.
  - Host combines the tiny per-core outputs in float64; counts n_k come
    from bincount (exact), center norms |C_k|^2 from numpy f64.
"""

import sys
from contextlib import ExitStack

import numpy as np

for _p in ("/opt/trn_rl_repo", "/opt/pypackages"):
    if _p not in sys.path:
        sys.path.append(_p)

import ml_dtypes
import concourse.tile as tile
from concourse import bacc, mybir
from concourse.bass_utils import run_bass_kernel_spmd

N, D, K = 1_000_000, 64, 100
ALPHA, BETA = 1.0, 1.0
N_CORES = 8
N_PER_CORE = N // N_CORES      # 125000
P = 128                        # SBUF partitions == samples per slot
SLOTS = 32                     # slots per tile
SPT = P * SLOTS                # samples per tile = 4096
NTILES = 34                    # capacity 139264 >= 125000 + 100*127 worst pad
NSLOTS = NTILES * SLOTS        # 1088
DVE_TILES = 16                 # tiles whose x^2 runs on DVE (rest on ACT)

_fp8 = mybir.dt.float8e3       # e3m4: max 15.5, 4 mantissa bits
_bf16 = mybir.dt.bfloat16
_f32 = mybir.dt.float32
FP8 = ml_dtypes.float8_e3m4


def build_nc(ntiles: int = NTILES):
    nc = bacc.Bacc()
    xr_d = nc.dram_tensor("xr", [ntiles, P, SLOTS, 2 * D], _fp8, kind="ExternalInput")
    gram_out = nc.dram_tensor("gram", [P, 2 * D], _f32, kind="ExternalOutput")
    ssum_out = nc.dram_tensor("ssums", [P, ntiles * SLOTS], _f32,
                              kind="ExternalOutput")

    nslots = ntiles * SLOTS
    with ExitStack() as ctx:
        tc = ctx.enter_context(tile.TileContext(nc))
        const_pool = ctx.enter_context(tc.tile_pool(name="const", bufs=1))
        xin = ctx.enter_context(tc.tile_pool(name="xin", bufs=4))
        scratch = ctx.enter_context(tc.tile_pool(name="scratch", bufs=2))
        psum = ctx.enter_context(tc.tile_pool(name="psum", bufs=1, space="PSUM"))

        ones_sb = const_pool.tile([P, 1], _fp8)
        nc.vector.memset(ones_sb[:], 1.0)
        ssums_sb = const_pool.tile([P, nslots], _f32)

        gram_ps = psum.tile([P, 2 * D], _f32)
        nbank = (nslots + 511) // 512
        ssum_ps = []
        for b in range(nbank):
            ssum_ps_b = psum.tile([P, min(512, nslots - 512 * b)], _f32,
                                  tag=f"ssum_ps{b}")
            ssum_ps.append(ssum_ps_b)

        for t in range(ntiles):
            xr_t = xin.tile([P, SLOTS, 2 * D], _fp8)
            if t % 3 == 0:
                nc.sync.dma_start(xr_t[:], xr_d[t, :, :, :])
            elif t % 3 == 1:
                nc.scalar.dma_start(xr_t[:], xr_d[t, :, :, :])
            else:
                nc.gpsimd.dma_start(xr_t[:], xr_d[t, :, :, :])

            for j in range(SLOTS):
                gs = t * SLOTS + j
                slot = xr_t[:, j, :]             # [128, 128] = [x | r]
                nc.tensor.matmul(
                    gram_ps[:],
                    slot,
                    slot,                        # full [x|r] Gram
                    start=(gs == 0),
                    stop=(gs == nslots - 1),
                    skip_group_check=True,
                )
                nc.tensor.matmul(
                    ssum_ps[gs // 512][:, gs % 512 : gs % 512 + 1],
                    slot,
                    ones_sb[:],
                    start=True,
                    stop=True,
                    skip_group_check=True,
                )

        gram_sb = const_pool.tile([P, 2 * D], _f32)
        nc.vector.tensor_copy(gram_sb[:], gram_ps[:])
        for b in range(nbank):
            w = min(512, nslots - 512 * b)
            nc.vector.tensor_copy(ssums_sb[:, 512 * b : 512 * b + w], ssum_ps[b][:])
        nc.sync.dma_start(gram_out[:, :], gram_sb[:])
        nc.sync.dma_start(ssum_out[:, :], ssums_sb[:])

    nc.compile()
    from ldw_dedup import dedup_ldweights

    dedup_ldweights(nc)
    return nc


def host_prepare(recon_x, x, cluster_assignments, ntiles: int = NTILES,
                 n_cores: int = N_CORES):
    """Sort by cluster, pad clusters to slot (128) boundaries, cast fp8."""
    n = x.shape[0]
    npc = n // n_cores
    x_np = np.asarray(x, dtype=np.float32).reshape(n_cores, npc, D)
    r_np = np.asarray(recon_x, dtype=np.float32).reshape(n_cores, npc, D)
    a_np = np.asarray(cluster_assignments, dtype=np.int64).reshape(n_cores, npc)

    cap = ntiles * SPT
    in_maps = []
    meta = []
    for c in range(n_cores):
        a = a_np[c]
        order = np.argsort(a, kind="stable")
        a_sorted = a[order]
        cnt = np.bincount(a, minlength=K)
        pad = ((cnt + P - 1) // P) * P
        starts = np.concatenate(([0], np.cumsum(cnt)))[:K]
        starts_pad = np.concatenate(([0], np.cumsum(pad)))
        total_pad = int(starts_pad[K])
        assert total_pad <= cap, (total_pad, cap)
        rank = np.arange(npc, dtype=np.int64) - starts[a_sorted]
        pos = starts_pad[a_sorted] + rank

        xp = np.zeros((cap, D), FP8)
        rp = np.zeros((cap, D), FP8)
        xp[pos] = x_np[c][order].astype(FP8)
        rp[pos] = r_np[c][order].astype(FP8)

        # [cap, D] -> [ntiles, 32, 128, D] -> [ntiles, 128, 32, D]
        xp4 = xp.reshape(ntiles, SLOTS, P, D).transpose(0, 2, 1, 3)
        rp4 = rp.reshape(ntiles, SLOTS, P, D).transpose(0, 2, 1, 3)
        xr = np.concatenate([xp4, rp4], axis=3)  # [ntiles, 128, 32, 128]
        in_maps.append({"xr": np.ascontiguousarray(xr)})

        nslot_k = pad // P
        slot_cluster = np.full(ntiles * SLOTS, -1, dtype=np.int64)
        filled = np.repeat(np.arange(K, dtype=np.int64), nslot_k)
        slot_cluster[: filled.shape[0]] = filled
        meta.append({"slot_cluster": slot_cluster, "cnt": cnt})
    return in_maps, meta


def host_combine(results, meta, cluster_centers, n_real: int = N):
    C = np.asarray(cluster_centers, dtype=np.float64)
    w = (C * C).sum(axis=1)                       # |C_k|^2

    xsq = 0.0
    rsq = 0.0
    cross_r = 0.0
    cross_c = 0.0
    wsum = 0.0
    for rd, md in zip(results, meta):
        gram = rd["gram"].astype(np.float64)      # [128, 128]
        d = np.arange(D)
        xsq += gram[d, d].sum()
        cross_r += gram[d, D + d].sum()
        rsq += gram[D + d, D + d].sum()

        ss = rd["ssums"].astype(np.float64)[:D]   # [64, nslots] x slot sums
        sc = md["slot_cluster"]
        valid = sc >= 0
        S = np.zeros((K, D))
        np.add.at(S, sc[valid], ss[:, valid].T)
        cross_c += (S * C).sum()
        wsum += (md["cnt"].astype(np.float64) * w).sum()

    recon = rsq - 2.0 * cross_r + xsq
    cluster = xsq - 2.0 * cross_c + wsum
    total = ALPHA * recon + BETA * cluster
    return (np.float32(total), np.float32(recon), np.float32(cluster))


_nc = None


def _get_nc():
    global _nc
    if _nc is None:
        _nc = build_nc()
    return _nc


def kernel(recon_x, x, cluster_assignments, cluster_centers):
    nc = _get_nc()
    in_maps, meta = host_prepare(recon_x, x, cluster_assignments)
    res = run_bass_kernel_spmd(nc, in_maps, list(range(N_CORES)))
    return host_combine(res.results, meta, cluster_centers)


# revision 16
# speedup vs baseline: 1.1577x; 1.1577x over previous
"""DeepClusterLoss on 8 Trainium2 NeuronCores (Bass/Tile).

reference:
    recon_loss   = sum((recon_x - x)**2)
    cluster_loss = sum((x - centers[assign])**2)
    total        = recon_loss + cluster_loss          (ALPHA = BETA = 1)

Strategy:
  - HOST index prep: sort ALL N samples by cluster assignment, pad each
    cluster's run to a multiple of 128 so every 128-sample "slot" belongs
    to one cluster (pads are zero rows), then deal slots round-robin to
    the 8 cores (pure layout work: permutation + fp8 cast, no arithmetic
    on the data).
  - Streams are fp8 e3m4 (max 15.5, 4 mantissa bits; N(0,1) data sums see
    ~1e-4 statistical bias, tolerance is 2e-2).  Layout per tile:
    [128, 32 pairs, 256] where a pair-block is [x_j | x_j' | r_j | r_j'].
  - Tensor engine, per slot-PAIR (one LDWEIGHTS of [x_j|x_j'], which is
    the measured bottleneck row):
      MM_Q: rhs = whole 256-col block -> [128,256] psum accumulated over
            all pairs: left diag = sum x^2, right diag = sum <x,r>.
      MM_S: rhs = ones -> [128,1] per-pair column: rows 0:64 = slot-j x
            sums, rows 64:128 = slot-j' x sums (segment sums; host groups
            them into S_k, no one-hot ever materialized).
  - sum r^2 runs on the otherwise-idle Scalar (Square + accum_out) and
    Vector (affine_mul_reduce) engines, split to balance.
  - Host combines per-core outputs in float64; counts n_k via bincount.
"""

import sys
from contextlib import ExitStack

import numpy as np

for _p in ("/opt/trn_rl_repo", "/opt/pypackages"):
    if _p not in sys.path:
        sys.path.append(_p)

import ml_dtypes
import concourse.tile as tile
from concourse import bacc, mybir
from concourse.bass_utils import run_bass_kernel_spmd

N, D, K = 1_000_000, 64, 100
ALPHA, BETA = 1.0, 1.0
N_CORES = 8
P = 128                        # SBUF partitions == samples per slot
SLOTS = 64                     # slots per tile (32 pair-blocks)
PAIRS = SLOTS // 2
SPT = P * SLOTS                # samples per tile = 8192
NTILES = 16                    # 16*64 = 1024 slots/core; worst need 989
NSLOTS = NTILES * SLOTS        # 1024
NPAIRS = NSLOTS // 2           # 512 (exactly one psum bank of columns)
DVE_TILES = 7                  # tiles whose r^2 runs on DVE (rest on ACT)

_fp8 = mybir.dt.float8e3       # e3m4: max 15.5, 4 mantissa bits
_bf16 = mybir.dt.bfloat16
_f32 = mybir.dt.float32
FP8 = ml_dtypes.float8_e3m4


def build_nc(ntiles: int = NTILES):
    nc = bacc.Bacc()
    xr_d = nc.dram_tensor("xr", [ntiles, P, PAIRS, 4 * D], _fp8,
                          kind="ExternalInput")
    quad_out = nc.dram_tensor("quad", [P, 4 * D], _f32, kind="ExternalOutput")
    ssum_out = nc.dram_tensor("ssums", [P, ntiles * PAIRS], _f32,
                              kind="ExternalOutput")
    part_out = nc.dram_tensor("partials", [P, ntiles], _f32,
                              kind="ExternalOutput")

    npairs = ntiles * PAIRS
    with ExitStack() as ctx:
        tc = ctx.enter_context(tile.TileContext(nc))
        const_pool = ctx.enter_context(tc.tile_pool(name="const", bufs=1))
        xin = ctx.enter_context(tc.tile_pool(name="xin", bufs=3))
        scratch = ctx.enter_context(tc.tile_pool(name="scratch", bufs=2))
        psum = ctx.enter_context(tc.tile_pool(name="psum", bufs=1, space="PSUM"))

        ones_sb = const_pool.tile([P, 1], _fp8)
        nc.vector.memset(ones_sb[:], 1.0)
        ssums_sb = const_pool.tile([P, npairs], _f32)
        partials_sb = const_pool.tile([P, ntiles], _f32)

        quad_ps = psum.tile([P, 4 * D], _f32)
        nbank = (npairs + 511) // 512
        ssum_ps = []
        for b in range(nbank):
            ssum_ps_b = psum.tile([P, min(512, npairs - 512 * b)], _f32,
                                  tag=f"ssum_ps{b}")
            ssum_ps.append(ssum_ps_b)

        for t in range(ntiles):
            xr_t = xin.tile([P, PAIRS, 4 * D], _fp8)
            if t % 2 == 0:
                nc.sync.dma_start(xr_t[:], xr_d[t, :, :, :])
            else:
                nc.scalar.dma_start(xr_t[:], xr_d[t, :, :, :])

            for u in range(PAIRS):
                gp = t * PAIRS + u
                xpair = xr_t[:, u, 0 : 2 * D]        # [128, 128] stationary
                nc.tensor.matmul(
                    quad_ps[:],
                    xpair,
                    xr_t[:, u, :],                   # full 256-col block
                    start=(gp == 0),
                    stop=(gp == npairs - 1),
                    skip_group_check=True,
                )
                nc.tensor.matmul(
                    ssum_ps[gp // 512][:, gp % 512 : gp % 512 + 1],
                    xpair,
                    ones_sb[:],
                    start=True,
                    stop=True,
                    skip_group_check=True,
                )

            rv = xr_t[:, :, 2 * D : 4 * D]           # [128, 32, 128] r view
            sq_t = scratch.tile([P, PAIRS, 2 * D], _bf16, tag="sq")
            if t < DVE_TILES:
                nc.vector.affine_mul_reduce(
                    sq_t[:], partials_sb[:, t : t + 1], rv, rv, 1.0, 0.0
                )
            else:
                nc.scalar.activation(
                    sq_t[:], rv, mybir.ActivationFunctionType.Square,
                    accum_out=partials_sb[:, t : t + 1],
                )

        quad_sb = const_pool.tile([P, 4 * D], _f32)
        nc.vector.tensor_copy(quad_sb[:], quad_ps[:])
        for b in range(nbank):
            w = min(512, npairs - 512 * b)
            nc.vector.tensor_copy(ssums_sb[:, 512 * b : 512 * b + w], ssum_ps[b][:])
        nc.sync.dma_start(quad_out[:, :], quad_sb[:])
        nc.sync.dma_start(ssum_out[:, :], ssums_sb[:])
        nc.sync.dma_start(part_out[:, :], partials_sb[:])

    nc.compile()
    from ldw_dedup import dedup_ldweights

    dedup_ldweights(nc)
    return nc


def host_prepare(recon_x, x, cluster_assignments, ntiles: int = NTILES,
                 n_cores: int = N_CORES):
    """Global cluster sort, pad clusters to slot (128) boundaries, deal
    slots round-robin to cores, lay out pair-blocks, cast to fp8."""
    n = x.shape[0]
    x_np = np.asarray(x, dtype=np.float32)
    r_np = np.asarray(recon_x, dtype=np.float32)
    a = np.asarray(cluster_assignments, dtype=np.int64)

    order = np.argsort(a, kind="stable")
    a_sorted = a[order]
    cnt = np.bincount(a, minlength=K)
    pad = ((cnt + P - 1) // P) * P
    starts = np.concatenate(([0], np.cumsum(cnt)))[:K]
    starts_pad = np.concatenate(([0], np.cumsum(pad)))
    total_slots = int(starts_pad[K] // P)
    cap_slots = n_cores * ntiles * SLOTS
    assert total_slots <= cap_slots, (total_slots, cap_slots)

    rank = np.arange(n, dtype=np.int64) - starts[a_sorted]
    pos = starts_pad[a_sorted] + rank

    xp = np.zeros((cap_slots * P, D), FP8)
    rp = np.zeros((cap_slots * P, D), FP8)
    xp[pos] = x_np[order].astype(FP8)
    rp[pos] = r_np[order].astype(FP8)
    xp = xp.reshape(cap_slots, P, D)
    rp = rp.reshape(cap_slots, P, D)

    slot_cluster_g = np.full(cap_slots, -1, dtype=np.int64)
    filled = np.repeat(np.arange(K, dtype=np.int64), pad // P)
    slot_cluster_g[: filled.shape[0]] = filled

    in_maps = []
    meta = []
    for c in range(n_cores):
        xs = xp[c::n_cores]                     # [1024, 128, 64]
        rs = rp[c::n_cores]
        # -> [ntiles, 32 pairs, 2 slots, 128, 64] -> [nt, 128, 32, 2, 64]
        x5 = xs.reshape(ntiles, PAIRS, 2, P, D).transpose(0, 3, 1, 2, 4)
        r5 = rs.reshape(ntiles, PAIRS, 2, P, D).transpose(0, 3, 1, 2, 4)
        x4 = x5.reshape(ntiles, P, PAIRS, 2 * D)   # [x_j | x_j']
        r4 = r5.reshape(ntiles, P, PAIRS, 2 * D)   # [r_j | r_j']
        xr = np.concatenate([x4, r4], axis=3)      # [nt, 128, 32, 256]
        in_maps.append({"xr": np.ascontiguousarray(xr)})
        meta.append({"slot_cluster": slot_cluster_g[c::n_cores]})
    return in_maps, {"per_core": meta, "cnt": cnt}


def host_combine(results, meta, cluster_centers):
    C = np.asarray(cluster_centers, dtype=np.float64)
    w = (C * C).sum(axis=1)                       # |C_k|^2

    xsq = 0.0
    rsq = 0.0
    cross_r = 0.0
    cross_c = 0.0
    d2 = np.arange(2 * D)
    for rd, md in zip(results, meta["per_core"]):
        quad = rd["quad"].astype(np.float64)      # [128, 256]
        xsq += quad[d2, d2].sum()
        cross_r += quad[d2, 2 * D + d2].sum()
        rsq += rd["partials"].astype(np.float64).sum()

        ss = rd["ssums"].astype(np.float64)       # [128, 512]
        sc = md["slot_cluster"]
        S = np.zeros((K, D))
        ev = sc[0::2]                             # slot j  -> rows 0:64
        od = sc[1::2]                             # slot j' -> rows 64:128
        ve = ev >= 0
        vo = od >= 0
        np.add.at(S, ev[ve], ss[:D, ve].T)
        np.add.at(S, od[vo], ss[D:, vo].T)
        cross_c += (S * C).sum()

    wsum = (meta["cnt"].astype(np.float64) * w).sum()
    recon = rsq - 2.0 * cross_r + xsq
    cluster = xsq - 2.0 * cross_c + wsum
    total = ALPHA * recon + BETA * cluster
    return (np.float32(total), np.float32(recon), np.float32(cluster))


_nc = None


def _get_nc():
    global _nc
    if _nc is None:
        _nc = build_nc()
    return _nc


def kernel(recon_x, x, cluster_assignments, cluster_centers):
    nc = _get_nc()
    in_maps, meta = host_prepare(recon_x, x, cluster_assignments)
    res = run_bass_kernel_spmd(nc, in_maps, list(range(N_CORES)))
    return host_combine(res.results, meta, cluster_centers)


# revision 19
# speedup vs baseline: 1.2459x; 1.0762x over previous
"""DeepClusterLoss on 8 Trainium2 NeuronCores (Bass/Tile).

reference:
    recon_loss   = sum((recon_x - x)**2)
    cluster_loss = sum((x - centers[assign])**2)
    total        = recon_loss + cluster_loss          (ALPHA = BETA = 1)

Strategy:
  - HOST index prep: sort ALL N samples by cluster assignment, pad each
    cluster's run to a multiple of 128 so every 128-sample "slot" belongs
    to one cluster (pads are zero rows), then deal slots round-robin to
    the 8 cores (pure layout work: permutation + fp8 cast, no arithmetic
    on the data).
  - Streams are fp8 e3m4 (max 15.5, 4 mantissa bits; N(0,1) data sums see
    ~1e-4 statistical bias, tolerance is 2e-2).  Layout per tile:
    [128, 32 pairs, 256] where a pair-block is [x_j | x_j' | r_j | r_j'].
  - Tensor engine, per slot-PAIR (one LDWEIGHTS of [x_j|x_j'], which is
    the measured bottleneck row):
      MM_Q: rhs = whole 256-col block -> [128,256] psum accumulated over
            all pairs: left diag = sum x^2, right diag = sum <x,r>.
      MM_S: rhs = ones -> [128,1] per-pair column: rows 0:64 = slot-j x
            sums, rows 64:128 = slot-j' x sums (segment sums; host groups
            them into S_k, no one-hot ever materialized).
  - sum r^2 runs on the otherwise-idle Scalar (Square + accum_out) and
    Vector (affine_mul_reduce) engines, split to balance.
  - Host combines per-core outputs in float64; counts n_k via bincount.
"""

import sys
from contextlib import ExitStack

import numpy as np

for _p in ("/opt/trn_rl_repo", "/opt/pypackages"):
    if _p not in sys.path:
        sys.path.append(_p)

import ml_dtypes
import concourse.tile as tile
from concourse import bacc, mybir
from concourse.bass_utils import run_bass_kernel_spmd

N, D, K = 1_000_000, 64, 100
ALPHA, BETA = 1.0, 1.0
N_CORES = 8
P = 128                        # SBUF partitions == samples per slot
SLOTS = 62                     # slots per tile (31 pair-blocks)
PAIRS = SLOTS // 2
SPT = P * SLOTS                # samples per tile = 7936
NTILES = 16                    # 16*62 = 992 slots/core; worst need 989
NSLOTS = NTILES * SLOTS        # 1024
NPAIRS = NSLOTS // 2           # 512 (exactly one psum bank of columns)
DVE_TILES = 7                  # tiles whose r^2 runs on DVE (rest on ACT)

_fp8 = mybir.dt.float8e3       # e3m4: max 15.5, 4 mantissa bits
_bf16 = mybir.dt.bfloat16
_f32 = mybir.dt.float32
FP8 = ml_dtypes.float8_e3m4


def build_nc(ntiles: int = NTILES):
    nc = bacc.Bacc()
    xr_d = nc.dram_tensor("xr", [ntiles, P, PAIRS, 4 * D], _fp8,
                          kind="ExternalInput")
    quad_out = nc.dram_tensor("quad", [P, 4 * D], _f32, kind="ExternalOutput")
    ssum_out = nc.dram_tensor("ssums", [P, ntiles * PAIRS], _f32,
                              kind="ExternalOutput")
    part_out = nc.dram_tensor("partials", [P, ntiles], _f32,
                              kind="ExternalOutput")

    npairs = ntiles * PAIRS
    with ExitStack() as ctx:
        tc = ctx.enter_context(tile.TileContext(nc))
        const_pool = ctx.enter_context(tc.tile_pool(name="const", bufs=1))
        xin = ctx.enter_context(tc.tile_pool(name="xin", bufs=4))
        scratch = ctx.enter_context(tc.tile_pool(name="scratch", bufs=2))
        psum = ctx.enter_context(tc.tile_pool(name="psum", bufs=1, space="PSUM"))

        ones_sb = const_pool.tile([P, 1], _fp8)
        nc.vector.memset(ones_sb[:], 1.0)
        ssums_sb = const_pool.tile([P, npairs], _f32)
        partials_sb = const_pool.tile([P, ntiles], _f32)

        quad_ps = psum.tile([P, 4 * D], _f32)
        nbank = (npairs + 511) // 512
        ssum_ps = []
        for b in range(nbank):
            ssum_ps_b = psum.tile([P, min(512, npairs - 512 * b)], _f32,
                                  tag=f"ssum_ps{b}")
            ssum_ps.append(ssum_ps_b)

        for t in range(ntiles):
            xr_t = xin.tile([P, PAIRS, 4 * D], _fp8)
            nc.sync.dma_start(xr_t[:], xr_d[t, :, :, :])

            for u in range(PAIRS):
                gp = t * PAIRS + u
                xpair = xr_t[:, u, 0 : 2 * D]        # [128, 128] stationary
                nc.tensor.matmul(
                    quad_ps[:],
                    xpair,
                    xr_t[:, u, :],                   # full 256-col block
                    start=(gp == 0),
                    stop=(gp == npairs - 1),
                    skip_group_check=True,
                )
                nc.tensor.matmul(
                    ssum_ps[gp // 512][:, gp % 512 : gp % 512 + 1],
                    xpair,
                    ones_sb[:],
                    start=True,
                    stop=True,
                    skip_group_check=True,
                )

            rv = xr_t[:, :, 2 * D : 4 * D]           # [128, 32, 128] r view
            sq_t = scratch.tile([P, PAIRS, 2 * D], _bf16, tag="sq")
            if t < DVE_TILES:
                nc.vector.affine_mul_reduce(
                    sq_t[:], partials_sb[:, t : t + 1], rv, rv, 1.0, 0.0
                )
            else:
                nc.scalar.activation(
                    sq_t[:], rv, mybir.ActivationFunctionType.Square,
                    accum_out=partials_sb[:, t : t + 1],
                )

        quad_sb = const_pool.tile([P, 4 * D], _f32)
        nc.vector.tensor_copy(quad_sb[:], quad_ps[:])
        for b in range(nbank):
            w = min(512, npairs - 512 * b)
            nc.vector.tensor_copy(ssums_sb[:, 512 * b : 512 * b + w], ssum_ps[b][:])
        nc.sync.dma_start(quad_out[:, :], quad_sb[:])
        nc.sync.dma_start(ssum_out[:, :], ssums_sb[:])
        nc.sync.dma_start(part_out[:, :], partials_sb[:])

    nc.compile()
    from ldw_dedup import dedup_ldweights

    dedup_ldweights(nc)
    return nc


def host_prepare(recon_x, x, cluster_assignments, ntiles: int = NTILES,
                 n_cores: int = N_CORES):
    """Global cluster sort, pad clusters to slot (128) boundaries, deal
    slots round-robin to cores, lay out pair-blocks, cast to fp8."""
    n = x.shape[0]
    x_np = np.asarray(x, dtype=np.float32)
    r_np = np.asarray(recon_x, dtype=np.float32)
    a = np.asarray(cluster_assignments, dtype=np.int64)

    order = np.argsort(a, kind="stable")
    a_sorted = a[order]
    cnt = np.bincount(a, minlength=K)
    pad = ((cnt + P - 1) // P) * P
    starts = np.concatenate(([0], np.cumsum(cnt)))[:K]
    starts_pad = np.concatenate(([0], np.cumsum(pad)))
    total_slots = int(starts_pad[K] // P)
    cap_slots = n_cores * ntiles * SLOTS
    assert total_slots <= cap_slots, (total_slots, cap_slots)

    rank = np.arange(n, dtype=np.int64) - starts[a_sorted]
    pos = starts_pad[a_sorted] + rank

    xp = np.zeros((cap_slots * P, D), FP8)
    rp = np.zeros((cap_slots * P, D), FP8)
    xp[pos] = x_np[order].astype(FP8)
    rp[pos] = r_np[order].astype(FP8)
    xp = xp.reshape(cap_slots, P, D)
    rp = rp.reshape(cap_slots, P, D)

    slot_cluster_g = np.full(cap_slots, -1, dtype=np.int64)
    filled = np.repeat(np.arange(K, dtype=np.int64), pad // P)
    slot_cluster_g[: filled.shape[0]] = filled

    in_maps = []
    meta = []
    for c in range(n_cores):
        xs = xp[c::n_cores]                     # [1024, 128, 64]
        rs = rp[c::n_cores]
        # -> [ntiles, 32 pairs, 2 slots, 128, 64] -> [nt, 128, 32, 2, 64]
        x5 = xs.reshape(ntiles, PAIRS, 2, P, D).transpose(0, 3, 1, 2, 4)
        r5 = rs.reshape(ntiles, PAIRS, 2, P, D).transpose(0, 3, 1, 2, 4)
        x4 = x5.reshape(ntiles, P, PAIRS, 2 * D)   # [x_j | x_j']
        r4 = r5.reshape(ntiles, P, PAIRS, 2 * D)   # [r_j | r_j']
        xr = np.concatenate([x4, r4], axis=3)      # [nt, 128, 32, 256]
        in_maps.append({"xr": np.ascontiguousarray(xr)})
        meta.append({"slot_cluster": slot_cluster_g[c::n_cores]})
    return in_maps, {"per_core": meta, "cnt": cnt}


def host_combine(results, meta, cluster_centers):
    C = np.asarray(cluster_centers, dtype=np.float64)
    w = (C * C).sum(axis=1)                       # |C_k|^2

    xsq = 0.0
    rsq = 0.0
    cross_r = 0.0
    cross_c = 0.0
    d2 = np.arange(2 * D)
    for rd, md in zip(results, meta["per_core"]):
        quad = rd["quad"].astype(np.float64)      # [128, 256]
        xsq += quad[d2, d2].sum()
        cross_r += quad[d2, 2 * D + d2].sum()
        rsq += rd["partials"].astype(np.float64).sum()

        ss = rd["ssums"].astype(np.float64)       # [128, 512]
        sc = md["slot_cluster"]
        S = np.zeros((K, D))
        ev = sc[0::2]                             # slot j  -> rows 0:64
        od = sc[1::2]                             # slot j' -> rows 64:128
        ve = ev >= 0
        vo = od >= 0
        np.add.at(S, ev[ve], ss[:D, ve].T)
        np.add.at(S, od[vo], ss[D:, vo].T)
        cross_c += (S * C).sum()

    wsum = (meta["cnt"].astype(np.float64) * w).sum()
    recon = rsq - 2.0 * cross_r + xsq
    cluster = xsq - 2.0 * cross_c + wsum
    total = ALPHA * recon + BETA * cluster
    return (np.float32(total), np.float32(recon), np.float32(cluster))


_nc = None


def _get_nc():
    global _nc
    if _nc is None:
        _nc = build_nc()
    return _nc


def kernel(recon_x, x, cluster_assignments, cluster_centers):
    nc = _get_nc()
    in_maps, meta = host_prepare(recon_x, x, cluster_assignments)
    res = run_bass_kernel_spmd(nc, in_maps, list(range(N_CORES)))
    return host_combine(res.results, meta, cluster_centers)


# revision 28
# speedup vs baseline: 1.4169x; 1.1372x over previous
"""DeepClusterLoss on 8 Trainium2 NeuronCores (Bass/Tile).

reference:
    recon_loss   = sum((recon_x - x)**2)
    cluster_loss = sum((x - centers[assign])**2)
    total        = recon_loss + cluster_loss          (ALPHA = BETA = 1)

Strategy:
  - HOST index prep: sort ALL N samples by cluster assignment, pad each
    cluster's run to a multiple of 128 so every 128-sample "slot" belongs
    to one cluster (pads are zero rows), then deal slots round-robin to
    the 8 cores (pure layout work: permutation + fp8 cast, no arithmetic
    on the data).
  - Streams are fp8 e3m4 (max 15.5, 4 mantissa bits; N(0,1) data sums see
    ~1e-4 statistical bias, tolerance is 2e-2).  Layout per tile:
    [128, 32 pairs, 256] where a pair-block is [x_j | x_j' | r_j | r_j'].
  - Tensor engine, per slot-PAIR (one LDWEIGHTS of [x_j|x_j'], which is
    the measured bottleneck row):
      MM_Q: rhs = whole 256-col block -> [128,256] psum accumulated over
            all pairs: left diag = sum x^2, right diag = sum <x,r>.
      MM_S: rhs = ones -> [128,1] per-pair column: rows 0:64 = slot-j x
            sums, rows 64:128 = slot-j' x sums (segment sums; host groups
            them into S_k, no one-hot ever materialized).
  - sum r^2 runs on the otherwise-idle Scalar (Square + accum_out) and
    Vector (affine_mul_reduce) engines, split to balance.
  - Host combines per-core outputs in float64; counts n_k via bincount.
"""

import sys
from contextlib import ExitStack

import numpy as np

for _p in ("/opt/trn_rl_repo", "/opt/pypackages"):
    if _p not in sys.path:
        sys.path.append(_p)

import ml_dtypes
import concourse.tile as tile
from concourse import bacc, mybir
from concourse.bass_utils import run_bass_kernel_spmd

N, D, K = 1_000_000, 64, 100
ALPHA, BETA = 1.0, 1.0
N_CORES = 8
P = 128                        # SBUF partitions == samples per slot
SLOTS = 62                     # slots per tile (31 pair-blocks)
PAIRS = SLOTS // 2
SPT = P * SLOTS                # samples per tile = 7936
NTILES = 16                    # 16*62 = 992 slots/core; worst need 989
NSLOTS = NTILES * SLOTS        # 1024
NPAIRS = NSLOTS // 2           # 496 (fits one psum bank of columns)
SPLIT = tuple(range(1, 16, 2))  # tiles whose x^2 runs on ACT/DVE, not PE

_fp8 = mybir.dt.float8e3       # e3m4: max 15.5, 4 mantissa bits
_bf16 = mybir.dt.bfloat16
_f32 = mybir.dt.float32
FP8 = ml_dtypes.float8_e3m4


def build_nc(ntiles: int = NTILES):
    nc = bacc.Bacc()
    xr_d = nc.dram_tensor("xr", [ntiles, P, PAIRS, 4 * D], _fp8,
                          kind="ExternalInput")
    quad_out = nc.dram_tensor("quad", [P, 4 * D], _f32, kind="ExternalOutput")
    ssum_out = nc.dram_tensor("ssums", [P, ntiles * PAIRS], _f32,
                              kind="ExternalOutput")
    part_out = nc.dram_tensor("partials", [P, 2 * ntiles], _f32,
                              kind="ExternalOutput")

    npairs = ntiles * PAIRS
    with ExitStack() as ctx:
        tc = ctx.enter_context(tile.TileContext(nc))
        const_pool = ctx.enter_context(tc.tile_pool(name="const", bufs=1))
        xin = ctx.enter_context(tc.tile_pool(name="xin", bufs=4))
        scratch = ctx.enter_context(tc.tile_pool(name="scratch", bufs=3))
        psum = ctx.enter_context(tc.tile_pool(name="psum", bufs=1, space="PSUM"))

        ones_sb = const_pool.tile([P, 1], _fp8)
        nc.vector.memset(ones_sb[:], 1.0)
        ssums_sb = const_pool.tile([P, npairs], _f32)
        partials_sb = const_pool.tile([P, 2 * ntiles], _f32)
        nc.vector.memset(partials_sb[:], 0.0)

        quad_ps = psum.tile([P, 4 * D], _f32)
        nbank = (npairs + 511) // 512
        ssum_ps = []
        for b in range(nbank):
            ssum_ps_b = psum.tile([P, min(512, npairs - 512 * b)], _f32,
                                  tag=f"ssum_ps{b}")
            ssum_ps.append(ssum_ps_b)

        split = [t for t in range(ntiles) if t in SPLIT]
        full = [t for t in range(ntiles) if t not in SPLIT]
        last_full_gp = full[-1] * PAIRS + PAIRS - 1
        unit_idx = 0  # alternates r^2 / x^2 units between ACT and DVE

        for t in range(ntiles):
            xr_t = xin.tile([P, PAIRS, 4 * D], _fp8)
            nc.sync.dma_start(xr_t[:], xr_d[t, :, :, :])
            is_split = t in SPLIT

            for u in range(PAIRS):
                gp = t * PAIRS + u
                xpair = xr_t[:, u, 0 : 2 * D]        # [128, 128] stationary
                if is_split:
                    nc.tensor.matmul(
                        quad_ps[:, 2 * D : 4 * D],
                        xpair,
                        xr_t[:, u, 2 * D : 4 * D],   # r-pair only (cross)
                        start=False,
                        stop=(gp == npairs - 1),
                        skip_group_check=True,
                    )
                else:
                    nc.tensor.matmul(
                        quad_ps[:],
                        xpair,
                        xr_t[:, u, :],               # full 256-col block
                        start=(gp == 0),
                        stop=(gp == last_full_gp),
                        skip_group_check=True,
                    )
                nc.tensor.matmul(
                    ssum_ps[gp // 512][:, gp % 512 : gp % 512 + 1],
                    xpair,
                    ones_sb[:],
                    start=True,
                    stop=True,
                    skip_group_check=True,
                )

            views = [(xr_t[:, :, 2 * D : 4 * D], t)]          # r^2 -> col t
            if is_split:
                views.append((xr_t[:, :, 0 : 2 * D], ntiles + t))  # x^2
            for vv, col in views:
                sq_t = scratch.tile([P, PAIRS, 2 * D], _bf16, tag="sq")
                if unit_idx % 2 == 0 or unit_idx == 21:
                    nc.scalar.activation(
                        sq_t[:], vv, mybir.ActivationFunctionType.Square,
                        accum_out=partials_sb[:, col : col + 1],
                    )
                else:
                    nc.vector.affine_mul_reduce(
                        sq_t[:], partials_sb[:, col : col + 1], vv, vv, 1.0, 0.0
                    )
                unit_idx += 1

        quad_sb = const_pool.tile([P, 4 * D], _f32)
        nc.vector.tensor_copy(quad_sb[:], quad_ps[:])
        for b in range(nbank):
            w = min(512, npairs - 512 * b)
            nc.vector.tensor_copy(ssums_sb[:, 512 * b : 512 * b + w], ssum_ps[b][:])
        nc.sync.dma_start(quad_out[:, :], quad_sb[:])
        nc.sync.dma_start(ssum_out[:, :], ssums_sb[:])
        nc.sync.dma_start(part_out[:, :], partials_sb[:])

    nc.compile()
    from ldw_dedup import dedup_ldweights

    dedup_ldweights(nc)
    return nc


def host_prepare(recon_x, x, cluster_assignments, ntiles: int = NTILES,
                 n_cores: int = N_CORES):
    """Global cluster sort, pad clusters to slot (128) boundaries, deal
    slots round-robin to cores, lay out pair-blocks, cast to fp8."""
    n = x.shape[0]
    x_np = np.asarray(x, dtype=np.float32)
    r_np = np.asarray(recon_x, dtype=np.float32)
    a = np.asarray(cluster_assignments, dtype=np.int64)

    order = np.argsort(a, kind="stable")
    a_sorted = a[order]
    cnt = np.bincount(a, minlength=K)
    pad = ((cnt + P - 1) // P) * P
    starts = np.concatenate(([0], np.cumsum(cnt)))[:K]
    starts_pad = np.concatenate(([0], np.cumsum(pad)))
    total_slots = int(starts_pad[K] // P)
    cap_slots = n_cores * ntiles * SLOTS
    assert total_slots <= cap_slots, (total_slots, cap_slots)

    rank = np.arange(n, dtype=np.int64) - starts[a_sorted]
    pos = starts_pad[a_sorted] + rank

    xp = np.zeros((cap_slots * P, D), FP8)
    rp = np.zeros((cap_slots * P, D), FP8)
    xp[pos] = x_np[order].astype(FP8)
    rp[pos] = r_np[order].astype(FP8)
    xp = xp.reshape(cap_slots, P, D)
    rp = rp.reshape(cap_slots, P, D)

    slot_cluster_g = np.full(cap_slots, -1, dtype=np.int64)
    filled = np.repeat(np.arange(K, dtype=np.int64), pad // P)
    slot_cluster_g[: filled.shape[0]] = filled

    in_maps = []
    meta = []
    for c in range(n_cores):
        xs = xp[c::n_cores]                     # [1024, 128, 64]
        rs = rp[c::n_cores]
        # -> [ntiles, 32 pairs, 2 slots, 128, 64] -> [nt, 128, 32, 2, 64]
        x5 = xs.reshape(ntiles, PAIRS, 2, P, D).transpose(0, 3, 1, 2, 4)
        r5 = rs.reshape(ntiles, PAIRS, 2, P, D).transpose(0, 3, 1, 2, 4)
        x4 = x5.reshape(ntiles, P, PAIRS, 2 * D)   # [x_j | x_j']
        r4 = r5.reshape(ntiles, P, PAIRS, 2 * D)   # [r_j | r_j']
        xr = np.concatenate([x4, r4], axis=3)      # [nt, 128, 32, 256]
        in_maps.append({"xr": np.ascontiguousarray(xr)})
        meta.append({"slot_cluster": slot_cluster_g[c::n_cores]})
    return in_maps, {"per_core": meta, "cnt": cnt}


def host_combine(results, meta, cluster_centers):
    C = np.asarray(cluster_centers, dtype=np.float64)
    w = (C * C).sum(axis=1)                       # |C_k|^2

    xsq = 0.0
    rsq = 0.0
    cross_r = 0.0
    cross_c = 0.0
    d2 = np.arange(2 * D)
    for rd, md in zip(results, meta["per_core"]):
        quad = rd["quad"].astype(np.float64)      # [128, 256]
        part = rd["partials"].astype(np.float64)  # [128, 2*ntiles]
        nt = part.shape[1] // 2
        xsq += quad[d2, d2].sum() + part[:, nt:].sum()
        cross_r += quad[d2, 2 * D + d2].sum()
        rsq += part[:, :nt].sum()

        ss = rd["ssums"].astype(np.float64)       # [128, 512]
        sc = md["slot_cluster"]
        S = np.zeros((K, D))
        ev = sc[0::2]                             # slot j  -> rows 0:64
        od = sc[1::2]                             # slot j' -> rows 64:128
        ve = ev >= 0
        vo = od >= 0
        np.add.at(S, ev[ve], ss[:D, ve].T)
        np.add.at(S, od[vo], ss[D:, vo].T)
        cross_c += (S * C).sum()

    wsum = (meta["cnt"].astype(np.float64) * w).sum()
    recon = rsq - 2.0 * cross_r + xsq
    cluster = xsq - 2.0 * cross_c + wsum
    total = ALPHA * recon + BETA * cluster
    return (np.float32(total), np.float32(recon), np.float32(cluster))


_nc = None


def _get_nc():
    global _nc
    if _nc is None:
        _nc = build_nc()
    return _nc


def kernel(recon_x, x, cluster_assignments, cluster_centers):
    nc = _get_nc()
    in_maps, meta = host_prepare(recon_x, x, cluster_assignments)
    res = run_bass_kernel_spmd(nc, in_maps, list(range(N_CORES)))
    return host_combine(res.results, meta, cluster_centers)


# revision 33
# speedup vs baseline: 1.4232x; 1.0045x over previous
"""DeepClusterLoss on 8 Trainium2 NeuronCores (Bass/Tile).

reference:
    recon_loss   = sum((recon_x - x)**2)
    cluster_loss = sum((x - centers[assign])**2)
    total        = recon_loss + cluster_loss          (ALPHA = BETA = 1)

Strategy:
  - HOST index prep: sort ALL N samples by cluster assignment, pad each
    cluster's run to a multiple of 128 so every 128-sample "slot" belongs
    to one cluster (pads are zero rows), then deal slots round-robin to
    the 8 cores (pure layout work: permutation + fp8 cast, no arithmetic
    on the data).
  - Streams are fp8 e3m4 (max 15.5, 4 mantissa bits; N(0,1) data sums see
    ~1e-4 statistical bias, tolerance is 2e-2).  Layout per tile:
    [128, 32 pairs, 256] where a pair-block is [x_j | x_j' | r_j | r_j'].
  - Tensor engine, per slot-PAIR (one LDWEIGHTS of [x_j|x_j'], which is
    the measured bottleneck row):
      MM_Q: rhs = whole 256-col block -> [128,256] psum accumulated over
            all pairs: left diag = sum x^2, right diag = sum <x,r>.
      MM_S: rhs = ones -> [128,1] per-pair column: rows 0:64 = slot-j x
            sums, rows 64:128 = slot-j' x sums (segment sums; host groups
            them into S_k, no one-hot ever materialized).
  - sum r^2 runs on the otherwise-idle Scalar (Square + accum_out) and
    Vector (affine_mul_reduce) engines, split to balance.
  - Host combines per-core outputs in float64; counts n_k via bincount.
"""

import sys
from contextlib import ExitStack

import numpy as np

for _p in ("/opt/trn_rl_repo", "/opt/pypackages"):
    if _p not in sys.path:
        sys.path.append(_p)

import ml_dtypes
import concourse.tile as tile
from concourse import bacc, mybir
from concourse.bass_utils import run_bass_kernel_spmd

N, D, K = 1_000_000, 64, 100
ALPHA, BETA = 1.0, 1.0
N_CORES = 8
P = 128                        # SBUF partitions == samples per slot
SLOTS = 62                     # slots per tile (31 pair-blocks)
PAIRS = SLOTS // 2
SPT = P * SLOTS                # samples per tile = 7936
NTILES = 16                    # 16*62 = 992 slots/core; worst need 989
NSLOTS = NTILES * SLOTS        # 1024
NPAIRS = NSLOTS // 2           # 496 (fits one psum bank of columns)
SPLIT = (2, 4, 7, 9, 12, 14)   # tiles whose x^2 runs on ACT/DVE, not PE

_fp8 = mybir.dt.float8e3       # e3m4: max 15.5, 4 mantissa bits
_bf16 = mybir.dt.bfloat16
_f32 = mybir.dt.float32
FP8 = ml_dtypes.float8_e3m4


def build_nc(ntiles: int = NTILES):
    nc = bacc.Bacc()
    xr_d = nc.dram_tensor("xr", [ntiles, P, PAIRS, 4 * D], _fp8,
                          kind="ExternalInput")
    quad_out = nc.dram_tensor("quad", [P, 4 * D], _f32, kind="ExternalOutput")
    ssum_out = nc.dram_tensor("ssums", [P, ntiles * PAIRS], _f32,
                              kind="ExternalOutput")
    part_out = nc.dram_tensor("partials", [P, 2 * ntiles], _f32,
                              kind="ExternalOutput")

    npairs = ntiles * PAIRS
    with ExitStack() as ctx:
        tc = ctx.enter_context(tile.TileContext(nc))
        const_pool = ctx.enter_context(tc.tile_pool(name="const", bufs=1))
        xin = ctx.enter_context(tc.tile_pool(name="xin", bufs=6))
        scratch = ctx.enter_context(tc.tile_pool(name="scratch", bufs=3))
        psum = ctx.enter_context(tc.tile_pool(name="psum", bufs=1, space="PSUM"))

        ones_sb = const_pool.tile([P, 1], _fp8)
        nc.vector.memset(ones_sb[:], 1.0)
        ssums_sb = const_pool.tile([P, npairs], _f32)
        partials_sb = const_pool.tile([P, 2 * ntiles], _f32)
        nc.vector.memset(partials_sb[:], 0.0)
        # warm the ACT Square table while the first DMA is in flight
        warm_sb = const_pool.tile([P, 1], _bf16)
        nc.scalar.activation(warm_sb[:], ones_sb[:],
                             mybir.ActivationFunctionType.Square)

        quad_ps = psum.tile([P, 4 * D], _f32)
        nbank = (npairs + 511) // 512
        ssum_ps = []
        for b in range(nbank):
            ssum_ps_b = psum.tile([P, min(512, npairs - 512 * b)], _f32,
                                  tag=f"ssum_ps{b}")
            ssum_ps.append(ssum_ps_b)

        split = [t for t in range(ntiles) if t in SPLIT]
        full = [t for t in range(ntiles) if t not in SPLIT]
        last_full_gp = full[-1] * PAIRS + PAIRS - 1
        unit_idx = 0  # alternates r^2 / x^2 units between ACT and DVE

        for t in range(ntiles):
            xr_t = xin.tile([P, PAIRS, 4 * D], _fp8)
            if t == 0:
                # chunked so PE can start after the first ~quarter arrives
                q = PAIRS // 4
                for c0 in range(0, PAIRS, q):
                    c1 = min(c0 + q, PAIRS)
                    nc.sync.dma_start(xr_t[:, c0:c1, :], xr_d[t, :, c0:c1, :])
            else:
                nc.sync.dma_start(xr_t[:], xr_d[t, :, :, :])
            is_split = t in SPLIT

            for u in range(PAIRS):
                gp = t * PAIRS + u
                xpair = xr_t[:, u, 0 : 2 * D]        # [128, 128] stationary
                if is_split:
                    nc.tensor.matmul(
                        quad_ps[:, 2 * D : 4 * D],
                        xpair,
                        xr_t[:, u, 2 * D : 4 * D],   # r-pair only (cross)
                        start=False,
                        stop=False,
                        skip_group_check=True,
                    )
                else:
                    nc.tensor.matmul(
                        quad_ps[:],
                        xpair,
                        xr_t[:, u, :],               # full 256-col block
                        start=(gp == 0),
                        stop=(gp == last_full_gp),
                        skip_group_check=True,
                    )
                nc.tensor.matmul(
                    ssum_ps[gp // 512][:, gp % 512 : gp % 512 + 1],
                    xpair,
                    ones_sb[:],
                    start=True,
                    stop=True,
                    skip_group_check=True,
                )

            views = [(xr_t[:, :, 2 * D : 4 * D], t)]          # r^2 -> col t
            if is_split:
                views.append((xr_t[:, :, 0 : 2 * D], ntiles + t))  # x^2
            for vv, col in views:
                sq_t = scratch.tile([P, PAIRS, 2 * D], _bf16, tag="sq")
                if unit_idx % 2 == 0 or unit_idx == 21:
                    nc.scalar.activation(
                        sq_t[:], vv, mybir.ActivationFunctionType.Square,
                        accum_out=partials_sb[:, col : col + 1],
                    )
                else:
                    nc.vector.affine_mul_reduce(
                        sq_t[:], partials_sb[:, col : col + 1], vv, vv, 1.0, 0.0
                    )
                unit_idx += 1

        quad_sb = const_pool.tile([P, 4 * D], _f32)
        nc.vector.tensor_copy(quad_sb[:], quad_ps[:])
        for b in range(nbank):
            w = min(512, npairs - 512 * b)
            nc.vector.tensor_copy(ssums_sb[:, 512 * b : 512 * b + w], ssum_ps[b][:])
        nc.sync.dma_start(quad_out[:, :], quad_sb[:])
        nc.sync.dma_start(ssum_out[:, :], ssums_sb[:])
        nc.sync.dma_start(part_out[:, :], partials_sb[:])

    nc.compile()
    from ldw_dedup import dedup_ldweights

    dedup_ldweights(nc)
    return nc


def host_prepare(recon_x, x, cluster_assignments, ntiles: int = NTILES,
                 n_cores: int = N_CORES):
    """Global cluster sort, pad clusters to slot (128) boundaries, deal
    slots round-robin to cores, lay out pair-blocks, cast to fp8."""
    n = x.shape[0]
    x_np = np.asarray(x, dtype=np.float32)
    r_np = np.asarray(recon_x, dtype=np.float32)
    a = np.asarray(cluster_assignments, dtype=np.int64)

    order = np.argsort(a, kind="stable")
    a_sorted = a[order]
    cnt = np.bincount(a, minlength=K)
    pad = ((cnt + P - 1) // P) * P
    starts = np.concatenate(([0], np.cumsum(cnt)))[:K]
    starts_pad = np.concatenate(([0], np.cumsum(pad)))
    total_slots = int(starts_pad[K] // P)
    cap_slots = n_cores * ntiles * SLOTS
    assert total_slots <= cap_slots, (total_slots, cap_slots)

    rank = np.arange(n, dtype=np.int64) - starts[a_sorted]
    pos = starts_pad[a_sorted] + rank

    xp = np.zeros((cap_slots * P, D), FP8)
    rp = np.zeros((cap_slots * P, D), FP8)
    xp[pos] = x_np[order].astype(FP8)
    rp[pos] = r_np[order].astype(FP8)
    xp = xp.reshape(cap_slots, P, D)
    rp = rp.reshape(cap_slots, P, D)

    slot_cluster_g = np.full(cap_slots, -1, dtype=np.int64)
    filled = np.repeat(np.arange(K, dtype=np.int64), pad // P)
    slot_cluster_g[: filled.shape[0]] = filled

    in_maps = []
    meta = []
    for c in range(n_cores):
        xs = xp[c::n_cores]                     # [1024, 128, 64]
        rs = rp[c::n_cores]
        # -> [ntiles, 32 pairs, 2 slots, 128, 64] -> [nt, 128, 32, 2, 64]
        x5 = xs.reshape(ntiles, PAIRS, 2, P, D).transpose(0, 3, 1, 2, 4)
        r5 = rs.reshape(ntiles, PAIRS, 2, P, D).transpose(0, 3, 1, 2, 4)
        x4 = x5.reshape(ntiles, P, PAIRS, 2 * D)   # [x_j | x_j']
        r4 = r5.reshape(ntiles, P, PAIRS, 2 * D)   # [r_j | r_j']
        xr = np.concatenate([x4, r4], axis=3)      # [nt, 128, 32, 256]
        in_maps.append({"xr": np.ascontiguousarray(xr)})
        meta.append({"slot_cluster": slot_cluster_g[c::n_cores]})
    return in_maps, {"per_core": meta, "cnt": cnt}


def host_combine(results, meta, cluster_centers):
    C = np.asarray(cluster_centers, dtype=np.float64)
    w = (C * C).sum(axis=1)                       # |C_k|^2

    xsq = 0.0
    rsq = 0.0
    cross_r = 0.0
    cross_c = 0.0
    d2 = np.arange(2 * D)
    for rd, md in zip(results, meta["per_core"]):
        quad = rd["quad"].astype(np.float64)      # [128, 256]
        part = rd["partials"].astype(np.float64)  # [128, 2*ntiles]
        nt = part.shape[1] // 2
        xsq += quad[d2, d2].sum() + part[:, nt:].sum()
        cross_r += quad[d2, 2 * D + d2].sum()
        rsq += part[:, :nt].sum()

        ss = rd["ssums"].astype(np.float64)       # [128, 512]
        sc = md["slot_cluster"]
        S = np.zeros((K, D))
        ev = sc[0::2]                             # slot j  -> rows 0:64
        od = sc[1::2]                             # slot j' -> rows 64:128
        ve = ev >= 0
        vo = od >= 0
        np.add.at(S, ev[ve], ss[:D, ve].T)
        np.add.at(S, od[vo], ss[D:, vo].T)
        cross_c += (S * C).sum()

    wsum = (meta["cnt"].astype(np.float64) * w).sum()
    recon = rsq - 2.0 * cross_r + xsq
    cluster = xsq - 2.0 * cross_c + wsum
    total = ALPHA * recon + BETA * cluster
    return (np.float32(total), np.float32(recon), np.float32(cluster))


_nc = None


def _get_nc():
    global _nc
    if _nc is None:
        _nc = build_nc()
    return _nc


def kernel(recon_x, x, cluster_assignments, cluster_centers):
    nc = _get_nc()
    in_maps, meta = host_prepare(recon_x, x, cluster_assignments)
    res = run_bass_kernel_spmd(nc, in_maps, list(range(N_CORES)))
    return host_combine(res.results, meta, cluster_centers)
